# revision 12
# baseline (speedup 1.0000x reference)
"""Dequantized mixed-sign int8 GEMM on 8 trn2 NeuronCores.

out = ((x - X_ZP) * X_SCALE) @ ((y - Y_ZP) * Y_SCALE)   [4096 x 4096 x 4096]

Strategy (fast path): the raw quantized values are 8-bit integers
(x int8, y uint8), so the GEMM runs as fp8e4m3 x fp8e4m3 with
perf_mode=DoubleRow (two fp8 weights per PE cell -> K=256 contraction
per matmul, ~2x the bf16 FLOP rate).  The operands are centered on host
(a = x, b = y - 128, both in [-128, 127]) before the fp8 cast to
minimize rounding error (|err| <= 4 per element, measured rel err
~1.4e-3 on the target data, tolerance 2e-2).  The exact zero-point
cross terms are rank-1 and are added back on host:

  (x+66)(y-160) = a*b - 32*a + 66*b - 2112    summed over K
  out = S*(a@b) + S*(-32*rowsum(a) + 66*colsum(b) - 2112*K)

On chip the kernel is a pure fp8 DoubleRow GEMM: the STATIONARY operand
is a [128, 2, 128] slice of y (so each weight load feeds 2 matmuls over
the two x halves, keeping LDWEIGHTS off the critical path), the moving
operand is a [128, 2, 512] slice of the SBUF-resident x, accumulating
into 8 concurrent [128, 512] fp32 PSUM chains; the dequant scale lands
in the PSUM->SBUF drain (split DVE/ACT).  Output tiles are [n, m]
(transposed), assembled on host.

Sharding: 4-way over M x 2-way over N (core (mi, nj)); each core gets
x[mi].T and y[:, nj] pre-tiled into contiguous DMA blocks and produces
a [1024, 2048] block.  Fallbacks for non-int8-range data: lossless bf16
repack (integer data) or fp32.
"""

import sys

if "/opt/trn_rl_repo" not in sys.path:
    sys.path.insert(0, "/opt/trn_rl_repo")

import numpy as np

X_SCALE, X_ZP = 0.03, -66.0
Y_SCALE, Y_ZP = 0.025, 160.0
OUT_SCALE = float(np.float32(X_SCALE) * np.float32(Y_SCALE))

M = K = N = 4096
MI, NJ = 4, 2  # core grid: M split x N split
M_SH, N_SH = M // MI, N // NJ  # 1024, 2048 per core
N_CORES = MI * NJ
NBW = 512  # n-block width (one PSUM bank of fp32)


def build(m_sh=M_SH, n_sh=N_SH, k=K, nbw=NBW):
    """Build + compile the per-core Bass program (SPMD: same NEFF on all cores)."""
    from concourse import bacc, mybir, tile

    f32, bf16 = mybir.dt.float32, mybir.dt.bfloat16
    kp = k // 128  # K tiles of 128
    mo_n = m_sh // 128  # M tiles of 128
    nb_n = n_sh // nbw  # N blocks

    nc = bacc.Bacc("TRN2", target_bir_lowering=False, debug=False)
    xt_d = nc.dram_tensor("xt", (k, m_sh), f32, kind="ExternalInput")
    y_d = nc.dram_tensor("y", (k, n_sh), f32, kind="ExternalInput")
    o_d = nc.dram_tensor("o", (m_sh, n_sh), f32, kind="ExternalOutput")

    with tile.TileContext(nc) as tc:
        with (
            tc.tile_pool(name="xstage", bufs=3) as xstage,
            tc.tile_pool(name="ystage", bufs=8) as ystage,
            tc.tile_pool(name="xbf", bufs=1) as xbfp,
            tc.tile_pool(name="ybf", bufs=2) as ybfp,
            tc.tile_pool(name="opool", bufs=4) as opool,
            tc.tile_pool(name="psum", bufs=1, space="PSUM") as psum,
        ):
            def load_y(nb, ko):
                ys = ystage.tile([128, nbw], f32, tag="ys")
                nc.sync.dma_start(
                    ys[:],
                    y_d.ap()[128 * ko : 128 * (ko + 1), nb * nbw : (nb + 1) * nbw],
                )
                yb = ybfp.tile([128, nbw], bf16, tag=f"y{ko}")
                nc.vector.tensor_scalar_add(yb[:], ys[:], -Y_ZP)
                return yb

            # K-tile-interleaved emission: the DMA stream delivers, per K
            # tile, first the y block-0 slice then the x.T slice, so the
            # PE (in-order) can start accumulating as data arrives instead
            # of waiting for the whole resident x to land.
            xbf = []
            ybs0 = []
            for ko in range(kp):
                ybs0.append(load_y(0, ko))
                xs = xstage.tile([128, m_sh], f32, tag="xs")
                nc.sync.dma_start(xs[:], xt_d.ap()[128 * ko : 128 * (ko + 1), :])
                xb = xbfp.tile([128, m_sh], bf16, tag=f"x{ko}")
                # ACT engine: out = Copy(in * 1 + 66); keeps the x-shift off
                # the DVE (which handles the y stream) and off GpSimd (slow).
                nc.scalar.activation(
                    xb[:], xs[:], mybir.ActivationFunctionType.Copy, bias=-X_ZP
                )
                xbf.append(xb)

            for nb in range(nb_n):
                ybs = ybs0 if nb == 0 else [load_y(nb, ko) for ko in range(kp)]

                # All mo_n output chains progress together (one PSUM bank
                # each), interleaved per K step, so every arriving K tile
                # immediately unlocks mo_n matmuls for the in-order PE.
                pss = [
                    psum.tile([128, nbw], f32, tag=f"ps{mo}", name=f"ps{mo}")
                    for mo in range(mo_n)
                ]
                for ko in range(kp):
                    for mo in range(mo_n):
                        nc.tensor.matmul(
                            pss[mo][:],
                            xbf[ko][:, 128 * mo : 128 * (mo + 1)],
                            ybs[ko][:],
                            start=(ko == 0),
                            stop=(ko == kp - 1),
                        )
                for mo in range(mo_n):
                    ot = opool.tile([128, nbw], f32, tag="ot")
                    nc.scalar.activation(
                        ot[:], pss[mo][:], mybir.ActivationFunctionType.Copy,
                        scale=OUT_SCALE,
                    )
                    nc.sync.dma_start(
                        o_d.ap()[128 * mo : 128 * (mo + 1), nb * nbw : (nb + 1) * nbw],
                        ot[:],
                    )

    nc.compile()
    return nc


def build_bf16(m_sh=M_SH, n_sh=N_SH, k=K, nbw=NBW):
    """bf16-input variant: shards arrive as raw bf16 (exact for the
    integer-valued quantized data), pre-tiled on the host so every DMA is
    one fully-contiguous block.  Zero-point shifts happen in-place on chip
    (y on DVE, x on ACT); the scale lands in the PSUM->SBUF copy, which is
    split across DVE/ACT by mo parity so the 8 PSUM banks drain on two
    engines at block boundaries."""
    from concourse import bacc, mybir, tile

    f32, bf16 = mybir.dt.float32, mybir.dt.bfloat16
    kp = k // 128
    mo_n = m_sh // 128
    nb_n = n_sh // nbw
    xg_n = kp // 2  # x groups: [128, 2, m_sh] (two K tiles per load)
    yg_n = kp // 4  # y groups: [128, 4, nbw] (four K tiles per load)

    nc = bacc.Bacc("TRN2", target_bir_lowering=False, debug=False)
    xt_d = nc.dram_tensor("xt", (xg_n, 128, 2, m_sh), bf16, kind="ExternalInput")
    y_d = nc.dram_tensor("y", (nb_n, yg_n, 128, 4, nbw), bf16, kind="ExternalInput")
    o_d = nc.dram_tensor("o", (mo_n, nb_n, 128, nbw), f32, kind="ExternalOutput")

    with tile.TileContext(nc) as tc:
        with (
            tc.tile_pool(name="xbf", bufs=1) as xbfp,
            tc.tile_pool(name="ybf", bufs=3) as ybfp,
            tc.tile_pool(name="opool", bufs=4) as opool,
            tc.tile_pool(name="psum", bufs=1, space="PSUM") as psum,
        ):
            # per-ko accessors: x_at[ko] -> (tile, j-index); y block 0 gets
            # its own per-ko map because its first group is split
            x_at = [None] * kp
            y0_at = [None] * kp

            def load_x(g, eng_dve):
                xb = xbfp.tile([128, 2, m_sh], bf16, tag=f"x{g}", name=f"x{g}")
                nc.sync.dma_start(xb[:], xt_d.ap()[g])
                # alternate the shift between DVE and ACT so neither engine
                # paces the startup stream alone
                if eng_dve:
                    nc.vector.tensor_scalar_add(xb[:], xb[:], -X_ZP)
                else:
                    nc.scalar.activation(
                        xb[:], xb[:], mybir.ActivationFunctionType.Copy, bias=-X_ZP
                    )
                x_at[2 * g] = (xb, 0)
                x_at[2 * g + 1] = (xb, 1)

            def load_y(nb):
                tiles = []
                for g in range(yg_n):
                    yb = ybfp.tile(
                        [128, 4, nbw], bf16, tag=f"y{g}", name=f"y{nb}_{g}"
                    )
                    nc.sync.dma_start(yb[:], y_d.ap()[nb, g])
                    nc.vector.tensor_scalar_add(yb[:], yb[:], -Y_ZP)
                    tiles.append(yb)
                return tiles

            def x_slice(ko, mo):  # lhsT [128, 128]
                xb, j = x_at[ko]
                return xb[:, j, 128 * mo : 128 * (mo + 1)]

            def y_slice(ybs, ko):  # rhs [128, nbw]
                if ybs is None:  # block 0: per-ko map with split first group
                    yb, j = y0_at[ko]
                    return yb[:, j, :]
                g, j = divmod(ko, 4)
                return ybs[g][:, j, :]

            # Startup stream, smallest-first: single-K-tile slices of the
            # first x/y groups load first so the very first matmul waits on
            # ~384KB instead of ~1MB, then interleave the rest of y block 0
            # with x so the in-order PE accumulates as data arrives.
            def load_y0_part(j0, j1, tag):
                yb = ybfp.tile([128, j1 - j0, nbw], bf16, tag=tag, name=tag)
                nc.sync.dma_start(yb[:], y_d.ap()[0, 0][:, j0:j1, :])
                nc.vector.tensor_scalar_add(yb[:], yb[:], -Y_ZP)
                for j in range(j0, j1):
                    y0_at[j] = (yb, j - j0)

            def load_x0_part(j, tag, eng_dve):
                xb = xbfp.tile([128, 1, m_sh], bf16, tag=tag, name=tag)
                nc.sync.dma_start(xb[:], xt_d.ap()[0][:, j : j + 1, :])
                if eng_dve:
                    nc.vector.tensor_scalar_add(xb[:], xb[:], -X_ZP)
                else:
                    nc.scalar.activation(
                        xb[:], xb[:], mybir.ActivationFunctionType.Copy, bias=-X_ZP
                    )
                x_at[j] = (xb, 0)

            load_y0_part(0, 1, "y0a")  # ko 0
            load_x0_part(0, "x0a", eng_dve=True)  # ko 0
            load_x0_part(1, "x0b", eng_dve=False)  # ko 1
            if kp > 1:
                load_y0_part(1, min(4, kp), "y0b")  # ko 1..3
            if xg_n > 1:
                load_x(1, eng_dve=True)  # ko 2..3
            for gg in range(1, yg_n):
                yb = ybfp.tile([128, 4, nbw], bf16, tag=f"y{gg}", name=f"y0_{gg}")
                nc.sync.dma_start(yb[:], y_d.ap()[0, gg])
                nc.vector.tensor_scalar_add(yb[:], yb[:], -Y_ZP)
                for j in range(4):
                    y0_at[4 * gg + j] = (yb, j)
                for g in (2 * gg, 2 * gg + 1):
                    if g < xg_n:
                        load_x(g, eng_dve=(g % 2 == 0))

            ybs_next = load_y(1) if nb_n > 1 else None
            ybs = None  # block 0 sentinel: y_slice uses the per-ko map
            for nb in range(nb_n):
                pss = [
                    psum.tile([128, nbw], f32, tag=f"ps{mo}", name=f"ps{mo}")
                    for mo in range(mo_n)
                ]

                def copy_out(mo, nb=nb):
                    ot = opool.tile([128, nbw], f32, tag="ot", name="ot")
                    # the very last chain's copy goes to DVE (0.55us) rather
                    # than ACT (2us): it sits on the kernel's critical tail
                    use_dve = mo % 2 == 0 or (
                        nb == nb_n - 1 and mo == mo_n - 1
                    )
                    if use_dve:
                        nc.vector.tensor_scalar_mul(ot[:], pss[mo][:], OUT_SCALE)
                    else:
                        nc.scalar.activation(
                            ot[:], pss[mo][:], mybir.ActivationFunctionType.Copy,
                            scale=OUT_SCALE,
                        )
                    nc.sync.dma_start(o_d.ap()[mo, nb], ot[:])

                if nb == 0:
                    # block 0 is paced by the input stream: interleave all
                    # chains per K step so every arriving K-tile group
                    # unlocks work for the in-order PE
                    for ko in range(kp):
                        for mo in range(mo_n):
                            nc.tensor.matmul(
                                pss[mo][:],
                                x_slice(ko, mo),
                                y_slice(ybs, ko),
                                start=(ko == 0),
                                stop=(ko == kp - 1),
                            )
                    for mo in range(mo_n):
                        copy_out(mo)
                else:
                    # data resident: run chains to completion one at a time
                    # so completions (and PSUM copies) stagger through the
                    # block instead of bursting at its end
                    for mo in range(mo_n):
                        for ko in range(kp):
                            nc.tensor.matmul(
                                pss[mo][:],
                                x_slice(ko, mo),
                                y_slice(ybs, ko),
                                start=(ko == 0),
                                stop=(ko == kp - 1),
                            )
                        copy_out(mo)
                ybs = ybs_next
                ybs_next = load_y(nb + 2) if nb + 2 < nb_n else None

    nc.compile()
    return nc


def build_fp8(m_sh=M_SH, n_sh=N_SH, k=K, nbw=NBW):
    """fp8e4m3 DoubleRow variant: centered operands arrive as fp8 (cast on
    host), zero-point terms are corrected on host, so the chip does a pure
    quantized GEMM at the fp8 DoubleRow rate.

    Layouts (per core):
      xt: [td, 128, 2, m_sh]        x.T in DoubleRow K-groups (K=256 each)
      y:  [nb, yg, 128, 4, nbw]     y in 4-K-tile groups per N block
      o:  [nb, ns, mh, 128, 512]    output.T tiles (n on partitions)

    Stationary operand = y slice [128, 2, 128] (one weight load feeds the
    mh_n moving x halves); moving = x slice [128, 2, 512].  Block 0 is
    emitted K-step-outer so the startup DMA stream unlocks matmuls as it
    arrives; later blocks run one n-slice pair at a time so PSUM drains
    stagger through the block."""
    from concourse import bacc, mybir, tile

    f32, fp8 = mybir.dt.float32, mybir.dt.float8e4
    DR = mybir.MatmulPerfMode.DoubleRow
    kp = k // 128  # 32 K tiles of 128
    td = kp // 2  # 16 DoubleRow K steps (K=256 each)
    yg_n = kp // 4  # y groups per N block (4 K tiles = 2 DR steps)
    nb_n = n_sh // nbw  # 4 N blocks
    ns_n = nbw // 128  # 4 stationary n-slices per block
    mh_n = m_sh // 512  # 2 moving m halves

    nc = bacc.Bacc("TRN2", target_bir_lowering=False, debug=False)
    xt_d = nc.dram_tensor("xt", (td, 128, 2, m_sh), fp8, kind="ExternalInput")
    y_d = nc.dram_tensor("y", (nb_n, yg_n, 128, 4, nbw), fp8, kind="ExternalInput")
    o_d = nc.dram_tensor(
        "o", (nb_n, ns_n, mh_n, 128, 512), f32, kind="ExternalOutput"
    )

    with tile.TileContext(nc) as tc:
        with (
            tc.tile_pool(name="xpool", bufs=1) as xp,
            tc.tile_pool(name="ypool", bufs=3) as yp,
            tc.tile_pool(name="opool", bufs=4) as op,
            tc.tile_pool(name="psum", bufs=1, space="PSUM") as psum,
        ):
            xts = [None] * td

            def load_x(t):
                xb = xp.tile([128, 2, m_sh], fp8, tag=f"x{t}", name=f"x{t}")
                nc.sync.dma_start(xb[:], xt_d.ap()[t])
                xts[t] = xb

            def load_y(nb):
                tiles = []
                for g in range(yg_n):
                    yb = yp.tile([128, 4, nbw], fp8, tag=f"y{g}", name=f"y{nb}_{g}")
                    nc.sync.dma_start(yb[:], y_d.ap()[nb, g])
                    tiles.append(yb)
                return tiles

            # block 0 y arrives as per-DR-step [128, 2, nbw] tiles so the
            # first matmuls wait on ~128KB, not a full 256KB group
            y0s = [None] * td

            def load_y0(t):
                g, j = divmod(t, 2)
                yb = yp.tile([128, 2, nbw], fp8, tag=f"y0_{t}", name=f"y0_{t}")
                nc.sync.dma_start(yb[:], y_d.ap()[0, g][:, 2 * j : 2 * j + 2, :])
                y0s[t] = yb

            # the very first K step lands in even smaller pieces (ns-0 y
            # slice + one x half) so matmul #1 waits on ~160KB and the HAM
            # warm-up window starts as early as possible
            y00 = [None, None]
            x0h = [None, None]

            def y_slice(ybs, t, ns):  # stationary [128, 2, 128]
                if ybs is None:  # block 0
                    if t == 0:
                        if ns == 0:
                            return y00[0][:]
                        return y00[1][:, :, 128 * (ns - 1) : 128 * ns]
                    return y0s[t][:, :, 128 * ns : 128 * (ns + 1)]
                g, j = divmod(t, 2)
                return ybs[g][:, 2 * j : 2 * j + 2, 128 * ns : 128 * (ns + 1)]

            def x_slice(t, mh):  # moving [128, 2, 512]
                if t == 0:
                    return x0h[mh][:]
                return xts[t][:, :, 512 * mh : 512 * (mh + 1)]

            # Startup stream: interleave y block-0 slices with x groups so
            # the in-order PE unlocks 8 matmuls per arriving K step.
            y00[0] = yp.tile([128, 2, 128], fp8, tag="y00a", name="y00a")
            nc.sync.dma_start(y00[0][:], y_d.ap()[0, 0][:, 0:2, 0:128])
            x0h[0] = xp.tile([128, 2, 512], fp8, tag="x0h0", name="x0h0")
            nc.sync.dma_start(x0h[0][:], xt_d.ap()[0][:, :, 0:512])
            y00[1] = yp.tile([128, 2, 384], fp8, tag="y00b", name="y00b")
            nc.sync.dma_start(y00[1][:], y_d.ap()[0, 0][:, 0:2, 128:512])
            x0h[1] = xp.tile([128, 2, 512], fp8, tag="x0h1", name="x0h1")
            nc.sync.dma_start(x0h[1][:], xt_d.ap()[0][:, :, 512:1024])
            for t in range(1, td):
                load_y0(t)
                load_x(t)
            ybs_next = load_y(1) if nb_n > 1 else None

            ybs = None  # block 0 sentinel
            for nb in range(nb_n):
                pss = [
                    [
                        psum.tile(
                            [128, 512], f32, tag=f"ps{ns}_{mh}", name=f"ps{ns}_{mh}"
                        )
                        for mh in range(mh_n)
                    ]
                    for ns in range(ns_n)
                ]

                def copy_out(ns, mh, dve=True, nb=nb, pss=pss):
                    # drain on DVE (0.55us vs 2us on ACT); issue the store
                    # from the ACT engine's hardware DMA queue so outputs
                    # never head-of-line-block the input stream on Sync
                    ot = op.tile([128, 512], f32, tag="ot", name="ot")
                    nc.vector.tensor_scalar_mul(ot[:], pss[ns][mh][:], OUT_SCALE)
                    nc.scalar.dma_start(o_d.ap()[nb, ns, mh], ot[:])

                def mm(ns, mh, t, ybs=ybs, pss=pss):
                    nc.tensor.matmul(
                        pss[ns][mh][:],
                        y_slice(ybs, t, ns),
                        x_slice(t, mh),
                        start=(t == 0),
                        stop=(t == td - 1),
                        perf_mode=DR,
                    )

                if nb == 0:
                    # K-step-outer: every arriving (y0[t], x[t]) pair unlocks
                    # 8 matmuls for the in-order PE during the DMA-paced start
                    for t in range(td):
                        for ns in range(ns_n):
                            for mh in range(mh_n):
                                mm(ns, mh, t)
                    # ns=0 drains first on the fast engine so block 1's first
                    # chains don't wait on a 2us ACT copy
                    for ns in range(ns_n):
                        for mh in range(mh_n):
                            copy_out(ns, mh, dve=(ns % 2 == 0))
                else:
                    last = nb == nb_n - 1
                    for ns in range(ns_n):
                        if last and ns == ns_n - 1:
                            # tail: serialize the two chains so mh=0's drain
                            # and store overlap mh=1's matmuls
                            for mh in range(mh_n):
                                for t in range(td):
                                    mm(ns, mh, t)
                                copy_out(ns, mh, dve=True)
                        else:
                            for t in range(td):
                                for mh in range(mh_n):
                                    mm(ns, mh, t)
                            for mh in range(mh_n):
                                copy_out(ns, mh, dve=(ns % 2 == 0))
                ybs = ybs_next
                ybs_next = load_y(nb + 2) if nb + 2 < nb_n else None

    nc.compile()
    return nc


_nc_cache = {}


def _get_nc(variant="f32"):
    if variant not in _nc_cache:
        if variant == "fp8":
            _nc_cache[variant] = build_fp8()
        elif variant == "bf16":
            _nc_cache[variant] = build_bf16()
        else:
            _nc_cache[variant] = build()
    return _nc_cache[variant]


def make_in_maps(x: np.ndarray, y: np.ndarray) -> list[dict]:
    x = np.ascontiguousarray(x, dtype=np.float32)
    y = np.ascontiguousarray(y, dtype=np.float32)
    xt_shards = [
        np.ascontiguousarray(x[mi * M_SH : (mi + 1) * M_SH].T) for mi in range(MI)
    ]
    y_shards = [
        np.ascontiguousarray(y[:, nj * N_SH : (nj + 1) * N_SH]) for nj in range(NJ)
    ]
    return [{"xt": xt_shards[i // NJ], "y": y_shards[i % NJ]} for i in range(N_CORES)]


def make_in_maps_bf16(xb: np.ndarray, yb: np.ndarray) -> list[dict]:
    """Pre-tile bf16 shards to match build_bf16's DRAM layouts.

    xt: [K, M_SH] -> [K/256, 128, 2, M_SH]   (contiguous 2-K-tile groups)
    y:  [K, N_SH] -> [NB, K/512, 128, 4, NBW] (contiguous 4-K-tile groups)
    """
    kp = K // 128
    nb_n = N_SH // NBW
    xt_shards = []
    for mi in range(MI):
        xt = xb[mi * M_SH : (mi + 1) * M_SH].T  # [K, M_SH]
        t = xt.reshape(kp // 2, 2, 128, M_SH).transpose(0, 2, 1, 3)
        xt_shards.append(np.ascontiguousarray(t))
    y_shards = []
    for nj in range(NJ):
        ys = yb[:, nj * N_SH : (nj + 1) * N_SH]  # [K, N_SH]
        t = ys.reshape(kp // 4, 4, 128, nb_n, NBW).transpose(3, 0, 2, 1, 4)
        y_shards.append(np.ascontiguousarray(t))
    return [{"xt": xt_shards[i // NJ], "y": y_shards[i % NJ]} for i in range(N_CORES)]


def _int8_range_ok(x: np.ndarray, y: np.ndarray) -> bool:
    """True when the inputs are the raw quantized integers this module
    targets (x int8-valued, y uint8-valued), making the fp8 path's error
    bound hold."""
    if not (np.array_equal(np.rint(x), x) and np.array_equal(np.rint(y), y)):
        return False
    return (
        x.min() >= -128 and x.max() <= 127 and y.min() >= 0 and y.max() <= 255
    )


def make_in_maps_fp8(x: np.ndarray, y: np.ndarray) -> list[dict]:
    """Center + cast to fp8e4m3 and pre-tile to build_fp8's DRAM layouts.

    a = x, b = y - 128 (both in [-128, 127], fp8 rounding err <= 4)
    xt: [K, M_SH] -> [K/256, 128, 2, M_SH]    (DoubleRow K groups)
    y:  [K, N_SH] -> [NB, K/512, 128, 4, NBW] (4-K-tile groups per block)
    """
    import ml_dtypes

    fp8 = ml_dtypes.float8_e4m3
    kp = K // 128
    nb_n = N_SH // NBW
    a8 = np.ascontiguousarray(x, dtype=np.float32).astype(fp8)
    b8 = (np.ascontiguousarray(y, dtype=np.float32) - np.float32(128.0)).astype(fp8)
    xt_shards = []
    for mi in range(MI):
        xt = a8[mi * M_SH : (mi + 1) * M_SH].T  # [K, M_SH]
        t = xt.reshape(kp // 2, 2, 128, M_SH).transpose(0, 2, 1, 3)
        xt_shards.append(np.ascontiguousarray(t))
    y_shards = []
    for nj in range(NJ):
        ys = b8[:, nj * N_SH : (nj + 1) * N_SH]  # [K, N_SH]
        t = ys.reshape(kp // 4, 4, 128, nb_n, NBW).transpose(3, 0, 2, 1, 4)
        y_shards.append(np.ascontiguousarray(t))
    return [{"xt": xt_shards[i // NJ], "y": y_shards[i % NJ]} for i in range(N_CORES)]


def assemble_fp8(results: list[dict], x: np.ndarray, y: np.ndarray) -> np.ndarray:
    """Gather per-core [nb, ns, mh, 128, 512] output.T tiles into the full
    [M, N] array and add the exact rank-1 zero-point correction:

    (x+66)(y-160) = a*b - 32*a + 66*b - 2112  with a = x, b = y-128."""
    out = np.empty((M, N), dtype=np.float32)
    for i in range(N_CORES):
        mi, nj = i // NJ, i % NJ
        o = results[i]["o"]  # [nb, ns, mh, n=128, m=512]
        blk = o.transpose(2, 4, 0, 1, 3).reshape(M_SH, N_SH)
        out[mi * M_SH : (mi + 1) * M_SH, nj * N_SH : (nj + 1) * N_SH] = blk
    rsa = x.sum(axis=1, dtype=np.float64)  # exact: integer sums < 2^53
    csb = (y.astype(np.float64) - 128.0).sum(axis=0)
    corr = -32.0 * rsa[:, None] + 66.0 * csb[None, :] + (-2112.0 * K)
    out += (OUT_SCALE * corr).astype(np.float32)
    return out


def _cast_bf16_exact(x: np.ndarray, y: np.ndarray):
    """Lossless repack to bf16 when every value survives the cast (true for
    the integer-valued quantized inputs this module targets)."""
    import ml_dtypes

    xb = np.ascontiguousarray(x, dtype=np.float32).astype(ml_dtypes.bfloat16)
    yb = np.ascontiguousarray(y, dtype=np.float32).astype(ml_dtypes.bfloat16)
    if np.array_equal(xb.astype(np.float32), x) and np.array_equal(
        yb.astype(np.float32), y
    ):
        return xb, yb
    return None


def kernel(x: np.ndarray, y: np.ndarray) -> np.ndarray:
    from concourse import bass_utils

    x = np.ascontiguousarray(x, dtype=np.float32)
    y = np.ascontiguousarray(y, dtype=np.float32)

    if _int8_range_ok(x, y):
        nc = _get_nc("fp8")
        in_maps = make_in_maps_fp8(x, y)
        res = bass_utils.run_bass_kernel_spmd(
            nc, in_maps, core_ids=list(range(N_CORES))
        )
        return assemble_fp8(res.results, x, y)

    casted = _cast_bf16_exact(x, y)
    if casted is not None:
        nc = _get_nc("bf16")
        in_maps = make_in_maps_bf16(*casted)
    else:  # rare fallback: data not exactly representable in bf16
        nc = _get_nc("f32")
        in_maps = make_in_maps(x, y)

    res = bass_utils.run_bass_kernel_spmd(nc, in_maps, core_ids=list(range(N_CORES)))

    out = np.empty((M, N), dtype=np.float32)
    for i in range(N_CORES):
        mi, nj = i // NJ, i % NJ
        o = res.results[i]["o"]
        if o.ndim == 4:  # [MO, NB, 128, NBW] pre-tiled layout
            o = o.transpose(0, 2, 1, 3).reshape(M_SH, N_SH)
        out[mi * M_SH : (mi + 1) * M_SH, nj * N_SH : (nj + 1) * N_SH] = o
    return out



# revision 13
# speedup vs baseline: 1.0015x; 1.0015x over previous
"""Dequantized mixed-sign int8 GEMM on 8 trn2 NeuronCores.

out = ((x - X_ZP) * X_SCALE) @ ((y - Y_ZP) * Y_SCALE)   [4096 x 4096 x 4096]

Strategy (fast path): the raw quantized values are 8-bit integers
(x int8, y uint8), so the GEMM runs as fp8e4m3 x fp8e4m3 with
perf_mode=DoubleRow (two fp8 weights per PE cell -> K=256 contraction
per matmul, ~2x the bf16 FLOP rate).  The operands are centered on host
(a = x, b = y - 128, both in [-128, 127]) before the fp8 cast to
minimize rounding error (|err| <= 4 per element, measured rel err
~1.4e-3 on the target data, tolerance 2e-2).  The exact zero-point
cross terms are rank-1 and are added back on host:

  (x+66)(y-160) = a*b - 32*a + 66*b - 2112    summed over K
  out = S*(a@b) + S*(-32*rowsum(a) + 66*colsum(b) - 2112*K)

On chip the kernel is a pure fp8 DoubleRow GEMM: the STATIONARY operand
is a [128, 2, 128] slice of y (so each weight load feeds 2 matmuls over
the two x halves, keeping LDWEIGHTS off the critical path), the moving
operand is a [128, 2, 512] slice of the SBUF-resident x, accumulating
into 8 concurrent [128, 512] fp32 PSUM chains; the dequant scale lands
in the PSUM->SBUF drain (split DVE/ACT).  Output tiles are [n, m]
(transposed), assembled on host.

Sharding: 4-way over M x 2-way over N (core (mi, nj)); each core gets
x[mi].T and y[:, nj] pre-tiled into contiguous DMA blocks and produces
a [1024, 2048] block.  Fallbacks for non-int8-range data: lossless bf16
repack (integer data) or fp32.
"""

import sys

if "/opt/trn_rl_repo" not in sys.path:
    sys.path.insert(0, "/opt/trn_rl_repo")

import numpy as np

X_SCALE, X_ZP = 0.03, -66.0
Y_SCALE, Y_ZP = 0.025, 160.0
OUT_SCALE = float(np.float32(X_SCALE) * np.float32(Y_SCALE))

M = K = N = 4096
MI, NJ = 4, 2  # core grid: M split x N split
M_SH, N_SH = M // MI, N // NJ  # 1024, 2048 per core
N_CORES = MI * NJ
NBW = 512  # n-block width (one PSUM bank of fp32)


def build(m_sh=M_SH, n_sh=N_SH, k=K, nbw=NBW):
    """Build + compile the per-core Bass program (SPMD: same NEFF on all cores)."""
    from concourse import bacc, mybir, tile

    f32, bf16 = mybir.dt.float32, mybir.dt.bfloat16
    kp = k // 128  # K tiles of 128
    mo_n = m_sh // 128  # M tiles of 128
    nb_n = n_sh // nbw  # N blocks

    nc = bacc.Bacc("TRN2", target_bir_lowering=False, debug=False)
    xt_d = nc.dram_tensor("xt", (k, m_sh), f32, kind="ExternalInput")
    y_d = nc.dram_tensor("y", (k, n_sh), f32, kind="ExternalInput")
    o_d = nc.dram_tensor("o", (m_sh, n_sh), f32, kind="ExternalOutput")

    with tile.TileContext(nc) as tc:
        with (
            tc.tile_pool(name="xstage", bufs=3) as xstage,
            tc.tile_pool(name="ystage", bufs=8) as ystage,
            tc.tile_pool(name="xbf", bufs=1) as xbfp,
            tc.tile_pool(name="ybf", bufs=2) as ybfp,
            tc.tile_pool(name="opool", bufs=4) as opool,
            tc.tile_pool(name="psum", bufs=1, space="PSUM") as psum,
        ):
            def load_y(nb, ko):
                ys = ystage.tile([128, nbw], f32, tag="ys")
                nc.sync.dma_start(
                    ys[:],
                    y_d.ap()[128 * ko : 128 * (ko + 1), nb * nbw : (nb + 1) * nbw],
                )
                yb = ybfp.tile([128, nbw], bf16, tag=f"y{ko}")
                nc.vector.tensor_scalar_add(yb[:], ys[:], -Y_ZP)
                return yb

            # K-tile-interleaved emission: the DMA stream delivers, per K
            # tile, first the y block-0 slice then the x.T slice, so the
            # PE (in-order) can start accumulating as data arrives instead
            # of waiting for the whole resident x to land.
            xbf = []
            ybs0 = []
            for ko in range(kp):
                ybs0.append(load_y(0, ko))
                xs = xstage.tile([128, m_sh], f32, tag="xs")
                nc.sync.dma_start(xs[:], xt_d.ap()[128 * ko : 128 * (ko + 1), :])
                xb = xbfp.tile([128, m_sh], bf16, tag=f"x{ko}")
                # ACT engine: out = Copy(in * 1 + 66); keeps the x-shift off
                # the DVE (which handles the y stream) and off GpSimd (slow).
                nc.scalar.activation(
                    xb[:], xs[:], mybir.ActivationFunctionType.Copy, bias=-X_ZP
                )
                xbf.append(xb)

            for nb in range(nb_n):
                ybs = ybs0 if nb == 0 else [load_y(nb, ko) for ko in range(kp)]

                # All mo_n output chains progress together (one PSUM bank
                # each), interleaved per K step, so every arriving K tile
                # immediately unlocks mo_n matmuls for the in-order PE.
                pss = [
                    psum.tile([128, nbw], f32, tag=f"ps{mo}", name=f"ps{mo}")
                    for mo in range(mo_n)
                ]
                for ko in range(kp):
                    for mo in range(mo_n):
                        nc.tensor.matmul(
                            pss[mo][:],
                            xbf[ko][:, 128 * mo : 128 * (mo + 1)],
                            ybs[ko][:],
                            start=(ko == 0),
                            stop=(ko == kp - 1),
                        )
                for mo in range(mo_n):
                    ot = opool.tile([128, nbw], f32, tag="ot")
                    nc.scalar.activation(
                        ot[:], pss[mo][:], mybir.ActivationFunctionType.Copy,
                        scale=OUT_SCALE,
                    )
                    nc.sync.dma_start(
                        o_d.ap()[128 * mo : 128 * (mo + 1), nb * nbw : (nb + 1) * nbw],
                        ot[:],
                    )

    nc.compile()
    return nc


def build_bf16(m_sh=M_SH, n_sh=N_SH, k=K, nbw=NBW):
    """bf16-input variant: shards arrive as raw bf16 (exact for the
    integer-valued quantized data), pre-tiled on the host so every DMA is
    one fully-contiguous block.  Zero-point shifts happen in-place on chip
    (y on DVE, x on ACT); the scale lands in the PSUM->SBUF copy, which is
    split across DVE/ACT by mo parity so the 8 PSUM banks drain on two
    engines at block boundaries."""
    from concourse import bacc, mybir, tile

    f32, bf16 = mybir.dt.float32, mybir.dt.bfloat16
    kp = k // 128
    mo_n = m_sh // 128
    nb_n = n_sh // nbw
    xg_n = kp // 2  # x groups: [128, 2, m_sh] (two K tiles per load)
    yg_n = kp // 4  # y groups: [128, 4, nbw] (four K tiles per load)

    nc = bacc.Bacc("TRN2", target_bir_lowering=False, debug=False)
    xt_d = nc.dram_tensor("xt", (xg_n, 128, 2, m_sh), bf16, kind="ExternalInput")
    y_d = nc.dram_tensor("y", (nb_n, yg_n, 128, 4, nbw), bf16, kind="ExternalInput")
    o_d = nc.dram_tensor("o", (mo_n, nb_n, 128, nbw), f32, kind="ExternalOutput")

    with tile.TileContext(nc) as tc:
        with (
            tc.tile_pool(name="xbf", bufs=1) as xbfp,
            tc.tile_pool(name="ybf", bufs=3) as ybfp,
            tc.tile_pool(name="opool", bufs=4) as opool,
            tc.tile_pool(name="psum", bufs=1, space="PSUM") as psum,
        ):
            # per-ko accessors: x_at[ko] -> (tile, j-index); y block 0 gets
            # its own per-ko map because its first group is split
            x_at = [None] * kp
            y0_at = [None] * kp

            def load_x(g, eng_dve):
                xb = xbfp.tile([128, 2, m_sh], bf16, tag=f"x{g}", name=f"x{g}")
                nc.sync.dma_start(xb[:], xt_d.ap()[g])
                # alternate the shift between DVE and ACT so neither engine
                # paces the startup stream alone
                if eng_dve:
                    nc.vector.tensor_scalar_add(xb[:], xb[:], -X_ZP)
                else:
                    nc.scalar.activation(
                        xb[:], xb[:], mybir.ActivationFunctionType.Copy, bias=-X_ZP
                    )
                x_at[2 * g] = (xb, 0)
                x_at[2 * g + 1] = (xb, 1)

            def load_y(nb):
                tiles = []
                for g in range(yg_n):
                    yb = ybfp.tile(
                        [128, 4, nbw], bf16, tag=f"y{g}", name=f"y{nb}_{g}"
                    )
                    nc.sync.dma_start(yb[:], y_d.ap()[nb, g])
                    nc.vector.tensor_scalar_add(yb[:], yb[:], -Y_ZP)
                    tiles.append(yb)
                return tiles

            def x_slice(ko, mo):  # lhsT [128, 128]
                xb, j = x_at[ko]
                return xb[:, j, 128 * mo : 128 * (mo + 1)]

            def y_slice(ybs, ko):  # rhs [128, nbw]
                if ybs is None:  # block 0: per-ko map with split first group
                    yb, j = y0_at[ko]
                    return yb[:, j, :]
                g, j = divmod(ko, 4)
                return ybs[g][:, j, :]

            # Startup stream, smallest-first: single-K-tile slices of the
            # first x/y groups load first so the very first matmul waits on
            # ~384KB instead of ~1MB, then interleave the rest of y block 0
            # with x so the in-order PE accumulates as data arrives.
            def load_y0_part(j0, j1, tag):
                yb = ybfp.tile([128, j1 - j0, nbw], bf16, tag=tag, name=tag)
                nc.sync.dma_start(yb[:], y_d.ap()[0, 0][:, j0:j1, :])
                nc.vector.tensor_scalar_add(yb[:], yb[:], -Y_ZP)
                for j in range(j0, j1):
                    y0_at[j] = (yb, j - j0)

            def load_x0_part(j, tag, eng_dve):
                xb = xbfp.tile([128, 1, m_sh], bf16, tag=tag, name=tag)
                nc.sync.dma_start(xb[:], xt_d.ap()[0][:, j : j + 1, :])
                if eng_dve:
                    nc.vector.tensor_scalar_add(xb[:], xb[:], -X_ZP)
                else:
                    nc.scalar.activation(
                        xb[:], xb[:], mybir.ActivationFunctionType.Copy, bias=-X_ZP
                    )
                x_at[j] = (xb, 0)

            load_y0_part(0, 1, "y0a")  # ko 0
            load_x0_part(0, "x0a", eng_dve=True)  # ko 0
            load_x0_part(1, "x0b", eng_dve=False)  # ko 1
            if kp > 1:
                load_y0_part(1, min(4, kp), "y0b")  # ko 1..3
            if xg_n > 1:
                load_x(1, eng_dve=True)  # ko 2..3
            for gg in range(1, yg_n):
                yb = ybfp.tile([128, 4, nbw], bf16, tag=f"y{gg}", name=f"y0_{gg}")
                nc.sync.dma_start(yb[:], y_d.ap()[0, gg])
                nc.vector.tensor_scalar_add(yb[:], yb[:], -Y_ZP)
                for j in range(4):
                    y0_at[4 * gg + j] = (yb, j)
                for g in (2 * gg, 2 * gg + 1):
                    if g < xg_n:
                        load_x(g, eng_dve=(g % 2 == 0))

            ybs_next = load_y(1) if nb_n > 1 else None
            ybs = None  # block 0 sentinel: y_slice uses the per-ko map
            for nb in range(nb_n):
                pss = [
                    psum.tile([128, nbw], f32, tag=f"ps{mo}", name=f"ps{mo}")
                    for mo in range(mo_n)
                ]

                def copy_out(mo, nb=nb):
                    ot = opool.tile([128, nbw], f32, tag="ot", name="ot")
                    # the very last chain's copy goes to DVE (0.55us) rather
                    # than ACT (2us): it sits on the kernel's critical tail
                    use_dve = mo % 2 == 0 or (
                        nb == nb_n - 1 and mo == mo_n - 1
                    )
                    if use_dve:
                        nc.vector.tensor_scalar_mul(ot[:], pss[mo][:], OUT_SCALE)
                    else:
                        nc.scalar.activation(
                            ot[:], pss[mo][:], mybir.ActivationFunctionType.Copy,
                            scale=OUT_SCALE,
                        )
                    nc.sync.dma_start(o_d.ap()[mo, nb], ot[:])

                if nb == 0:
                    # block 0 is paced by the input stream: interleave all
                    # chains per K step so every arriving K-tile group
                    # unlocks work for the in-order PE
                    for ko in range(kp):
                        for mo in range(mo_n):
                            nc.tensor.matmul(
                                pss[mo][:],
                                x_slice(ko, mo),
                                y_slice(ybs, ko),
                                start=(ko == 0),
                                stop=(ko == kp - 1),
                            )
                    for mo in range(mo_n):
                        copy_out(mo)
                else:
                    # data resident: run chains to completion one at a time
                    # so completions (and PSUM copies) stagger through the
                    # block instead of bursting at its end
                    for mo in range(mo_n):
                        for ko in range(kp):
                            nc.tensor.matmul(
                                pss[mo][:],
                                x_slice(ko, mo),
                                y_slice(ybs, ko),
                                start=(ko == 0),
                                stop=(ko == kp - 1),
                            )
                        copy_out(mo)
                ybs = ybs_next
                ybs_next = load_y(nb + 2) if nb + 2 < nb_n else None

    nc.compile()
    return nc


def build_fp8(m_sh=M_SH, n_sh=N_SH, k=K, nbw=NBW):
    """fp8e4m3 DoubleRow variant: centered operands arrive as fp8 (cast on
    host), zero-point terms are corrected on host, so the chip does a pure
    quantized GEMM at the fp8 DoubleRow rate.

    Layouts (per core):
      xt: [td, 128, 2, m_sh]        x.T in DoubleRow K-groups (K=256 each)
      y:  [nb, yg, 128, 4, nbw]     y in 4-K-tile groups per N block
      o:  [nb, ns, mh, 128, 512]    output.T tiles (n on partitions)

    Stationary operand = y slice [128, 2, 128] (one weight load feeds the
    mh_n moving x halves); moving = x slice [128, 2, 512].  Block 0 is
    emitted K-step-outer so the startup DMA stream unlocks matmuls as it
    arrives; later blocks run one n-slice pair at a time so PSUM drains
    stagger through the block."""
    from concourse import bacc, mybir, tile

    f32, fp8 = mybir.dt.float32, mybir.dt.float8e4
    DR = mybir.MatmulPerfMode.DoubleRow
    kp = k // 128  # 32 K tiles of 128
    td = kp // 2  # 16 DoubleRow K steps (K=256 each)
    yg_n = kp // 4  # y groups per N block (4 K tiles = 2 DR steps)
    nb_n = n_sh // nbw  # 4 N blocks
    ns_n = nbw // 128  # 4 stationary n-slices per block
    mh_n = m_sh // 512  # 2 moving m halves

    nc = bacc.Bacc("TRN2", target_bir_lowering=False, debug=False)
    xt_d = nc.dram_tensor("xt", (td, 128, 2, m_sh), fp8, kind="ExternalInput")
    y_d = nc.dram_tensor("y", (nb_n, yg_n, 128, 4, nbw), fp8, kind="ExternalInput")
    o_d = nc.dram_tensor(
        "o", (nb_n, ns_n, mh_n, 128, 512), f32, kind="ExternalOutput"
    )

    with tile.TileContext(nc) as tc:
        with (
            tc.tile_pool(name="xpool", bufs=1) as xp,
            tc.tile_pool(name="ypool", bufs=3) as yp,
            tc.tile_pool(name="opool", bufs=4) as op,
            tc.tile_pool(name="psum", bufs=1, space="PSUM") as psum,
        ):
            xts = [None] * td

            def load_x(t):
                xb = xp.tile([128, 2, m_sh], fp8, tag=f"x{t}", name=f"x{t}")
                nc.sync.dma_start(xb[:], xt_d.ap()[t])
                xts[t] = xb

            def load_y(nb):
                tiles = []
                for g in range(yg_n):
                    yb = yp.tile([128, 4, nbw], fp8, tag=f"y{g}", name=f"y{nb}_{g}")
                    nc.sync.dma_start(yb[:], y_d.ap()[nb, g])
                    tiles.append(yb)
                return tiles

            # block 0 y arrives as per-DR-step [128, 2, nbw] tiles so the
            # first matmuls wait on ~128KB, not a full 256KB group
            y0s = [None] * td

            def load_y0(t):
                g, j = divmod(t, 2)
                yb = yp.tile([128, 2, nbw], fp8, tag=f"y0_{t}", name=f"y0_{t}")
                nc.sync.dma_start(yb[:], y_d.ap()[0, g][:, 2 * j : 2 * j + 2, :])
                y0s[t] = yb

            # the very first K step lands in even smaller pieces (ns-0 y
            # slice + one x half) so matmul #1 waits on ~160KB and the HAM
            # warm-up window starts as early as possible
            y00 = [None, None]
            x0h = [None, None]

            def y_slice(ybs, t, ns):  # stationary [128, 2, 128]
                if ybs is None:  # block 0
                    if t == 0:
                        if ns == 0:
                            return y00[0][:]
                        return y00[1][:, :, 128 * (ns - 1) : 128 * ns]
                    return y0s[t][:, :, 128 * ns : 128 * (ns + 1)]
                g, j = divmod(t, 2)
                return ybs[g][:, 2 * j : 2 * j + 2, 128 * ns : 128 * (ns + 1)]

            def x_slice(t, mh):  # moving [128, 2, 512]
                if t == 0:
                    return x0h[mh][:]
                return xts[t][:, :, 512 * mh : 512 * (mh + 1)]

            # Startup stream: interleave y block-0 slices with x groups so
            # the in-order PE unlocks 8 matmuls per arriving K step.
            y00[0] = yp.tile([128, 2, 128], fp8, tag="y00a", name="y00a")
            nc.sync.dma_start(y00[0][:], y_d.ap()[0, 0][:, 0:2, 0:128])
            x0h[0] = xp.tile([128, 2, 512], fp8, tag="x0h0", name="x0h0")
            nc.sync.dma_start(x0h[0][:], xt_d.ap()[0][:, :, 0:512])
            y00[1] = yp.tile([128, 2, 384], fp8, tag="y00b", name="y00b")
            nc.sync.dma_start(y00[1][:], y_d.ap()[0, 0][:, 0:2, 128:512])
            x0h[1] = xp.tile([128, 2, 512], fp8, tag="x0h1", name="x0h1")
            nc.sync.dma_start(x0h[1][:], xt_d.ap()[0][:, :, 512:1024])
            for t in range(1, td):
                load_y0(t)
                load_x(t)
            ybs_next = load_y(1) if nb_n > 1 else None

            ybs = None  # block 0 sentinel
            for nb in range(nb_n):
                pss = [
                    [
                        psum.tile(
                            [128, 512], f32, tag=f"ps{ns}_{mh}", name=f"ps{ns}_{mh}"
                        )
                        for mh in range(mh_n)
                    ]
                    for ns in range(ns_n)
                ]

                def copy_out(ns, mh, dve=True, nb=nb, pss=pss):
                    # drain on DVE (0.55us vs 2us on ACT); issue the store
                    # from the ACT engine's hardware DMA queue so outputs
                    # never head-of-line-block the input stream on Sync
                    ot = op.tile([128, 512], f32, tag="ot", name="ot")
                    nc.vector.tensor_scalar_mul(ot[:], pss[ns][mh][:], OUT_SCALE)
                    nc.scalar.dma_start(o_d.ap()[nb, ns, mh], ot[:])

                def mm(ns, mh, t, ybs=ybs, pss=pss):
                    nc.tensor.matmul(
                        pss[ns][mh][:],
                        y_slice(ybs, t, ns),
                        x_slice(t, mh),
                        start=(t == 0),
                        stop=(t == td - 1),
                        perf_mode=DR,
                    )

                if nb == 0:
                    # K-step-outer: every arriving (y0[t], x[t]) pair unlocks
                    # 8 matmuls for the in-order PE during the DMA-paced start
                    for t in range(td):
                        for ns in range(ns_n):
                            for mh in range(mh_n):
                                mm(ns, mh, t)
                    # ns=0 drains first on the fast engine so block 1's first
                    # chains don't wait on a 2us ACT copy
                    for ns in range(ns_n):
                        for mh in range(mh_n):
                            copy_out(ns, mh, dve=(ns % 2 == 0))
                else:
                    last = nb == nb_n - 1
                    for ns in range(ns_n):
                        if last and ns == ns_n - 1:
                            # tail: serialize the two chains so mh=0's drain
                            # and store overlap mh=1's matmuls; the very
                            # last chain drains as two halves pushed through
                            # both hardware DMA queues in parallel
                            for t in range(td):
                                mm(ns, 0, t)
                            copy_out(ns, 0)
                            for t in range(td):
                                mm(ns, 1, t)
                            for half, eng in ((0, nc.scalar), (1, nc.sync)):
                                ot = op.tile([128, 256], f32, tag=f"otl{half}",
                                             name=f"otl{half}")
                                nc.vector.tensor_scalar_mul(
                                    ot[:],
                                    pss[ns][1][:, 256 * half : 256 * (half + 1)],
                                    OUT_SCALE,
                                )
                                eng.dma_start(
                                    o_d.ap()[nb, ns, 1][
                                        :, 256 * half : 256 * (half + 1)
                                    ],
                                    ot[:],
                                )
                        else:
                            for t in range(td):
                                for mh in range(mh_n):
                                    mm(ns, mh, t)
                            for mh in range(mh_n):
                                copy_out(ns, mh, dve=(ns % 2 == 0))
                ybs = ybs_next
                ybs_next = load_y(nb + 2) if nb + 2 < nb_n else None

    nc.compile()
    return nc


_nc_cache = {}


def _get_nc(variant="f32"):
    if variant not in _nc_cache:
        if variant == "fp8":
            _nc_cache[variant] = build_fp8()
        elif variant == "bf16":
            _nc_cache[variant] = build_bf16()
        else:
            _nc_cache[variant] = build()
    return _nc_cache[variant]


def make_in_maps(x: np.ndarray, y: np.ndarray) -> list[dict]:
    x = np.ascontiguousarray(x, dtype=np.float32)
    y = np.ascontiguousarray(y, dtype=np.float32)
    xt_shards = [
        np.ascontiguousarray(x[mi * M_SH : (mi + 1) * M_SH].T) for mi in range(MI)
    ]
    y_shards = [
        np.ascontiguousarray(y[:, nj * N_SH : (nj + 1) * N_SH]) for nj in range(NJ)
    ]
    return [{"xt": xt_shards[i // NJ], "y": y_shards[i % NJ]} for i in range(N_CORES)]


def make_in_maps_bf16(xb: np.ndarray, yb: np.ndarray) -> list[dict]:
    """Pre-tile bf16 shards to match build_bf16's DRAM layouts.

    xt: [K, M_SH] -> [K/256, 128, 2, M_SH]   (contiguous 2-K-tile groups)
    y:  [K, N_SH] -> [NB, K/512, 128, 4, NBW] (contiguous 4-K-tile groups)
    """
    kp = K // 128
    nb_n = N_SH // NBW
    xt_shards = []
    for mi in range(MI):
        xt = xb[mi * M_SH : (mi + 1) * M_SH].T  # [K, M_SH]
        t = xt.reshape(kp // 2, 2, 128, M_SH).transpose(0, 2, 1, 3)
        xt_shards.append(np.ascontiguousarray(t))
    y_shards = []
    for nj in range(NJ):
        ys = yb[:, nj * N_SH : (nj + 1) * N_SH]  # [K, N_SH]
        t = ys.reshape(kp // 4, 4, 128, nb_n, NBW).transpose(3, 0, 2, 1, 4)
        y_shards.append(np.ascontiguousarray(t))
    return [{"xt": xt_shards[i // NJ], "y": y_shards[i % NJ]} for i in range(N_CORES)]


def _int8_range_ok(x: np.ndarray, y: np.ndarray) -> bool:
    """True when the inputs are the raw quantized integers this module
    targets (x int8-valued, y uint8-valued), making the fp8 path's error
    bound hold."""
    if not (np.array_equal(np.rint(x), x) and np.array_equal(np.rint(y), y)):
        return False
    return (
        x.min() >= -128 and x.max() <= 127 and y.min() >= 0 and y.max() <= 255
    )


def make_in_maps_fp8(x: np.ndarray, y: np.ndarray) -> list[dict]:
    """Center + cast to fp8e4m3 and pre-tile to build_fp8's DRAM layouts.

    a = x, b = y - 128 (both in [-128, 127], fp8 rounding err <= 4)
    xt: [K, M_SH] -> [K/256, 128, 2, M_SH]    (DoubleRow K groups)
    y:  [K, N_SH] -> [NB, K/512, 128, 4, NBW] (4-K-tile groups per block)
    """
    import ml_dtypes

    fp8 = ml_dtypes.float8_e4m3
    kp = K // 128
    nb_n = N_SH // NBW
    a8 = np.ascontiguousarray(x, dtype=np.float32).astype(fp8)
    b8 = (np.ascontiguousarray(y, dtype=np.float32) - np.float32(128.0)).astype(fp8)
    xt_shards = []
    for mi in range(MI):
        xt = a8[mi * M_SH : (mi + 1) * M_SH].T  # [K, M_SH]
        t = xt.reshape(kp // 2, 2, 128, M_SH).transpose(0, 2, 1, 3)
        xt_shards.append(np.ascontiguousarray(t))
    y_shards = []
    for nj in range(NJ):
        ys = b8[:, nj * N_SH : (nj + 1) * N_SH]  # [K, N_SH]
        t = ys.reshape(kp // 4, 4, 128, nb_n, NBW).transpose(3, 0, 2, 1, 4)
        y_shards.append(np.ascontiguousarray(t))
    return [{"xt": xt_shards[i // NJ], "y": y_shards[i % NJ]} for i in range(N_CORES)]


def assemble_fp8(results: list[dict], x: np.ndarray, y: np.ndarray) -> np.ndarray:
    """Gather per-core [nb, ns, mh, 128, 512] output.T tiles into the full
    [M, N] array and add the exact rank-1 zero-point correction:

    (x+66)(y-160) = a*b - 32*a + 66*b - 2112  with a = x, b = y-128."""
    out = np.empty((M, N), dtype=np.float32)
    for i in range(N_CORES):
        mi, nj = i // NJ, i % NJ
        o = results[i]["o"]  # [nb, ns, mh, n=128, m=512]
        blk = o.transpose(2, 4, 0, 1, 3).reshape(M_SH, N_SH)
        out[mi * M_SH : (mi + 1) * M_SH, nj * N_SH : (nj + 1) * N_SH] = blk
    rsa = x.sum(axis=1, dtype=np.float64)  # exact: integer sums < 2^53
    csb = (y.astype(np.float64) - 128.0).sum(axis=0)
    corr = -32.0 * rsa[:, None] + 66.0 * csb[None, :] + (-2112.0 * K)
    out += (OUT_SCALE * corr).astype(np.float32)
    return out


def _cast_bf16_exact(x: np.ndarray, y: np.ndarray):
    """Lossless repack to bf16 when every value survives the cast (true for
    the integer-valued quantized inputs this module targets)."""
    import ml_dtypes

    xb = np.ascontiguousarray(x, dtype=np.float32).astype(ml_dtypes.bfloat16)
    yb = np.ascontiguousarray(y, dtype=np.float32).astype(ml_dtypes.bfloat16)
    if np.array_equal(xb.astype(np.float32), x) and np.array_equal(
        yb.astype(np.float32), y
    ):
        return xb, yb
    return None


def kernel(x: np.ndarray, y: np.ndarray) -> np.ndarray:
    from concourse import bass_utils

    x = np.ascontiguousarray(x, dtype=np.float32)
    y = np.ascontiguousarray(y, dtype=np.float32)

    if _int8_range_ok(x, y):
        nc = _get_nc("fp8")
        in_maps = make_in_maps_fp8(x, y)
        res = bass_utils.run_bass_kernel_spmd(
            nc, in_maps, core_ids=list(range(N_CORES))
        )
        return assemble_fp8(res.results, x, y)

    casted = _cast_bf16_exact(x, y)
    if casted is not None:
        nc = _get_nc("bf16")
        in_maps = make_in_maps_bf16(*casted)
    else:  # rare fallback: data not exactly representable in bf16
        nc = _get_nc("f32")
        in_maps = make_in_maps(x, y)

    res = bass_utils.run_bass_kernel_spmd(nc, in_maps, core_ids=list(range(N_CORES)))

    out = np.empty((M, N), dtype=np.float32)
    for i in range(N_CORES):
        mi, nj = i // NJ, i % NJ
        o = res.results[i]["o"]
        if o.ndim == 4:  # [MO, NB, 128, NBW] pre-tiled layout
            o = o.transpose(0, 2, 1, 3).reshape(M_SH, N_SH)
        out[mi * M_SH : (mi + 1) * M_SH, nj * N_SH : (nj + 1) * N_SH] = o
    return out

